# revision 4
# baseline (speedup 1.0000x reference)
"""v4: per-code indirect gathers against a host-split table. Narrow codes
(0..13) fetch only their 768B half-row (channels 0:4 or 4:8); wide codes
(14..22) fetch the full 1536B split row. Wide codes are gathered first so the
slower ACT consumers drain early and the tail is short. fp16 data-only device
output; host does the 0.5-fill + fp32 upcast during unshard.
"""

import numpy as np

import concourse.bacc as bacc
import concourse.bass as bass
import concourse.mybir as mybir
import concourse.tile as tile

BATCH = 8192
XCOLS = 512
NCODE = 23
NNARROW = 14
NBITS = 22
L = 131072
HROW = 2 * 48 * 4    # 384 f16 = 768B half-row
ROW = 2 * HROW       # 768 f16 = 1536B split row [lo, hi]
NCORES = 8
BC = BATCH // NCORES
P = 128
GROUPS = BC // P
DROW = 2 * 48 * 128  # 12288
Q = 96               # merged (p, k) = 2*48

f16 = mybir.dt.float16
f32 = mybir.dt.float32
i32 = mybir.dt.int32

N_SWDGE_QUEUES = 2


def build_module():
    nc = bacc.Bacc(
        "TRN2", target_bir_lowering=False, debug=False,
        num_swdge_queues=N_SWDGE_QUEUES,
    )
    x_t = nc.dram_tensor("x", [BC, XCOLS], i32, kind="ExternalInput")
    tab_t = nc.dram_tensor("table2", [2 * L, HROW], f16, kind="ExternalInput")
    tabw = tab_t[:].rearrange("(l h) c -> l (h c)", h=2)
    w_t = nc.dram_tensor("w", [P, NCODE * NBITS], f32, kind="ExternalInput")
    out_t = nc.dram_tensor("out", [BC, DROW], f16, kind="ExternalOutput")

    with tile.TileContext(nc) as tc:
        with (
            tc.tile_pool(name="const", bufs=1) as cpool,
            tc.tile_pool(name="xp", bufs=2) as xpool,
            tc.tile_pool(name="sm", bufs=GROUPS) as spool,
            tc.tile_pool(name="gn", bufs=28) as gnpool,
            tc.tile_pool(name="gw", bufs=18) as gwpool,
            tc.tile_pool(name="op", bufs=2) as opool,
        ):
            # first x load ahead of the constants so group 0's decode chain
            # starts as early as possible
            x_tiles = []
            x0 = xpool.tile([P, XCOLS], i32, tag="x0")
            nc.sync.dma_start(x0[:], x_t[0:P, :])
            w_tile = cpool.tile([P, NCODE * NBITS], f32)
            nc.sync.dma_start(w_tile[:], w_t[:])
            halfsel = cpool.tile([P, NNARROW], i32)
            nc.vector.memset(halfsel[:, 0:7], 0)
            nc.vector.memset(halfsel[:, 7:NNARROW], 1)

            idx2s, idxs, tts, sgs = [], [], [], []
            for g in range(GROUPS):
                b0 = g * P
                if g == 0:
                    x_tile = x0
                else:
                    x_tile = xpool.tile([P, XCOLS], i32)
                    nc.sync.dma_start(x_tile[:], x_t[b0 : b0 + P, :])
                xf = xpool.tile([P, XCOLS], f32)
                nc.vector.tensor_copy(out=xf[:], in_=x_tile[:])
                prod = xpool.tile([P, NCODE * NBITS], f32)
                nc.vector.tensor_tensor(
                    out=prod[:], in0=xf[:, 6:], in1=w_tile[:],
                    op=mybir.AluOpType.mult,
                )
                codes = spool.tile([P, NCODE], f32, tag="codes")
                nc.vector.tensor_reduce(
                    out=codes[:],
                    in_=prod[:].rearrange("n (c a) -> n c a", a=NBITS),
                    axis=mybir.AxisListType.X,
                    op=mybir.AluOpType.add,
                )
                codesi = spool.tile([P, NCODE], i32, tag="codesi")
                nc.vector.tensor_copy(out=codesi[:], in_=codes[:])
                idx = spool.tile([P, NCODE], i32, tag="idx")
                nc.vector.tensor_scalar(
                    out=idx[:], in0=codesi[:],
                    scalar1=L - 1, scalar2=None,
                    op0=mybir.AluOpType.bitwise_and,
                )
                idx2 = spool.tile([P, NNARROW], i32, tag="idx2")
                nc.vector.tensor_scalar(
                    out=idx2[:], in0=idx[:, 0:NNARROW],
                    scalar1=2, scalar2=None,
                    op0=mybir.AluOpType.mult,
                )
                nc.vector.tensor_tensor(
                    out=idx2[:], in0=idx2[:], in1=halfsel[:],
                    op=mybir.AluOpType.add,
                )
                tt = spool.tile([P, NCODE], f32, tag="tt")
                nc.vector.tensor_scalar(
                    out=tt[:], in0=codes[:],
                    scalar1=float(L), scalar2=None,
                    op0=mybir.AluOpType.is_gt,
                )
                sg = spool.tile([P, NCODE], f32, tag="sg")
                nc.vector.tensor_scalar(
                    out=sg[:], in0=tt[:],
                    scalar1=-2.0, scalar2=1.0,
                    op0=mybir.AluOpType.mult, op1=mybir.AluOpType.add,
                )
                idx2s.append(idx2); idxs.append(idx); tts.append(tt); sgs.append(sg)

            for g in range(GROUPS):
                b0 = g * P
                idx2, idx, tt, sg = idx2s[g], idxs[g], tts[g], sgs[g]
                od = opool.tile([P, DROW], f16)
                od3 = od[:].rearrange("n (q v) -> n q v", q=Q)
                odw = od[:].rearrange("n (q s h c) -> n s h q c", s=16, h=2, c=4)
                qi = 0
                # wide codes first: ACT consumers are the slow ones; issuing
                # their gathers first shortens the end-of-group tail.
                for c in range(NNARROW, NCODE):
                    gcw = gwpool.tile([P, ROW], f16)
                    gi = nc.gpsimd.indirect_dma_start(
                        out=gcw[:],
                        out_offset=None,
                        in_=tabw,
                        in_offset=bass.IndirectOffsetOnAxis(
                            ap=idx[:, c : c + 1], axis=0
                        ),
                    )
                    if N_SWDGE_QUEUES > 1 and qi % N_SWDGE_QUEUES:
                        gi.ins.queue = f"qPoolDynamic{qi % N_SWDGE_QUEUES}"
                    qi += 1
                    gvw = gcw[:].rearrange("n (h q c) -> n h q c", h=2, q=Q)
                    nc.scalar.activation(
                        out=odw[:, c - 7, :, :, :],
                        in_=gvw[:],
                        func=mybir.ActivationFunctionType.Identity,
                        bias=tt[:, c : c + 1],
                        scale=sg[:, c : c + 1],
                    )
                for c in range(NNARROW):
                    gcn = gnpool.tile([P, HROW], f16)
                    gi = nc.gpsimd.indirect_dma_start(
                        out=gcn[:],
                        out_offset=None,
                        in_=tab_t[:],
                        in_offset=bass.IndirectOffsetOnAxis(
                            ap=idx2[:, c : c + 1], axis=0
                        ),
                    )
                    if N_SWDGE_QUEUES > 1 and qi % N_SWDGE_QUEUES:
                        gi.ins.queue = f"qPoolDynamic{qi % N_SWDGE_QUEUES}"
                    qi += 1
                    gvn = gcn[:].rearrange("n (q c) -> n q c", q=Q)
                    col0 = c * 8 if c < 7 else (c - 7) * 8 + 4
                    nc.vector.tensor_scalar(
                        out=od3[:, :, col0 : col0 + 4],
                        in0=gvn[:],
                        scalar1=sg[:, c : c + 1],
                        scalar2=tt[:, c : c + 1],
                        op0=mybir.AluOpType.mult,
                        op1=mybir.AluOpType.add,
                    )
                eng = nc.sync if g % 2 == 0 else nc.scalar
                eng.dma_start(out=out_t[b0 : b0 + P, :], in_=od[:])
    nc.compile()
    return nc


def make_weights():
    w = np.tile((2.0 ** np.arange(NBITS)).astype(np.float32), NCODE)
    return np.broadcast_to(w, (P, NCODE * NBITS)).copy()


def split_table(table):
    t4 = np.ascontiguousarray(table).reshape(L, 2, 48, 8)
    tab2 = np.empty((L, 2, 2, 48, 4), dtype=np.float16)
    tab2[:, 0] = t4[:, :, :, 0:4]
    tab2[:, 1] = t4[:, :, :, 4:8]
    return tab2.reshape(2 * L, HROW)


def make_in_maps(x, table):
    tab2 = split_table(table)
    w = make_weights()
    return [
        {
            "x": np.ascontiguousarray(x[i * BC : (i + 1) * BC]),
            "table2": tab2,
            "w": w,
        }
        for i in range(NCORES)
    ]


def assemble_output(parts):
    out = np.full((BATCH, 2, 126, 128), 0.5, dtype=np.float32)
    for i, p in enumerate(parts):
        out[i * BC : (i + 1) * BC, :, 19:67, :] = p.reshape(BC, 2, 48, 128)
    return out


_NC_CACHE = None


def _get_module():
    global _NC_CACHE
    if _NC_CACHE is None:
        _NC_CACHE = build_module()
    return _NC_CACHE


def kernel(x: np.ndarray, table: np.ndarray) -> np.ndarray:
    from concourse.bass_utils import run_bass_kernel_spmd

    x = np.asarray(x)
    table = np.asarray(table)
    assert x.shape == (BATCH, XCOLS) and table.shape == (L, 2, 48, 8)
    nc = _get_module()
    res = run_bass_kernel_spmd(nc, make_in_maps(x, table), core_ids=list(range(NCORES)))
    return assemble_output([res.results[i]["out"] for i in range(NCORES)])


# revision 8
# speedup vs baseline: 1.0140x; 1.0140x over previous
"""v3 fallback: per-code indirect gathers (one offset per partition — the
lowering the HW ucode verifiably supports), fp16 data-only device output,
host-side 0.5-fill + fp32 upcast during unshard.
"""

import numpy as np

import concourse.bacc as bacc
import concourse.bass as bass
import concourse.mybir as mybir
import concourse.tile as tile

BATCH = 8192
XCOLS = 512
NCODE = 23
NBITS = 22
L = 131072
ROW = 2 * 48 * 8     # 768 fp16 = 1536B per table row
NCORES = 8
BC = BATCH // NCORES
P = 128
GROUPS = BC // P
DROW = 2 * 48 * 128  # 12288

f16 = mybir.dt.float16
f32 = mybir.dt.float32
i32 = mybir.dt.int32

N_SWDGE_QUEUES = 2


def _code_map(c):
    if c < 7:
        return 0, 4, c * 8
    if c < 14:
        return 4, 4, (c - 7) * 8 + 4
    return 0, 8, (c - 7) * 8


def build_module():
    nc = bacc.Bacc(
        "TRN2", target_bir_lowering=False, debug=False,
        num_swdge_queues=N_SWDGE_QUEUES,
    )
    x_t = nc.dram_tensor("x", [BC, XCOLS], i32, kind="ExternalInput")
    tab_t = nc.dram_tensor("table", [L, ROW], f16, kind="ExternalInput")
    w_t = nc.dram_tensor("w", [P, NCODE * NBITS], f32, kind="ExternalInput")
    out_t = nc.dram_tensor("out", [BC, DROW], f16, kind="ExternalOutput")

    with tile.TileContext(nc) as tc:
        with (
            tc.tile_pool(name="const", bufs=1) as cpool,
            tc.tile_pool(name="xp", bufs=2) as xpool,
            tc.tile_pool(name="sm", bufs=GROUPS) as spool,
            tc.tile_pool(name="gt", bufs=16) as gpool,
            tc.tile_pool(name="op", bufs=2) as opool,
        ):
            # group-0 x load ahead of the weights so the first decode chain
            # (which gates the first gather) starts as early as possible
            x0 = xpool.tile([P, XCOLS], i32, tag="x0")
            nc.sync.dma_start(x0[:], x_t[0:P, :])
            w_tile = cpool.tile([P, NCODE * NBITS], f32)
            nc.sync.dma_start(w_tile[:], w_t[:])

            idxs, tts, sgs = [], [], []
            for g in range(GROUPS):
                b0 = g * P
                if g == 0:
                    x_tile = x0
                else:
                    x_tile = xpool.tile([P, XCOLS], i32)
                    nc.sync.dma_start(x_tile[:], x_t[b0 : b0 + P, :])
                xf = xpool.tile([P, XCOLS], f32)
                nc.vector.tensor_copy(out=xf[:], in_=x_tile[:])
                prod = xpool.tile([P, NCODE * NBITS], f32)
                nc.vector.tensor_tensor(
                    out=prod[:], in0=xf[:, 6:], in1=w_tile[:],
                    op=mybir.AluOpType.mult,
                )
                codes = spool.tile([P, NCODE], f32, tag="codes")
                nc.vector.tensor_reduce(
                    out=codes[:],
                    in_=prod[:].rearrange("n (c a) -> n c a", a=NBITS),
                    axis=mybir.AxisListType.X,
                    op=mybir.AluOpType.add,
                )
                codesi = spool.tile([P, NCODE], i32, tag="codesi")
                nc.vector.tensor_copy(out=codesi[:], in_=codes[:])
                idx = spool.tile([P, NCODE], i32, tag="idx")
                nc.vector.tensor_scalar(
                    out=idx[:], in0=codesi[:],
                    scalar1=L - 1, scalar2=None,
                    op0=mybir.AluOpType.bitwise_and,
                )
                tt = spool.tile([P, NCODE], f32, tag="tt")
                nc.vector.tensor_scalar(
                    out=tt[:], in0=codes[:],
                    scalar1=float(L), scalar2=None,
                    op0=mybir.AluOpType.is_gt,
                )
                sg = spool.tile([P, NCODE], f32, tag="sg")
                nc.vector.tensor_scalar(
                    out=sg[:], in0=tt[:],
                    scalar1=-2.0, scalar2=1.0,
                    op0=mybir.AluOpType.mult, op1=mybir.AluOpType.add,
                )
                idxs.append(idx); tts.append(tt); sgs.append(sg)

            for g in range(GROUPS):
                b0 = g * P
                idx, tt, sg = idxs[g], tts[g], sgs[g]
                od = opool.tile([P, DROW], f16)
                od4 = od[:].rearrange("n (p k c) -> n p k c", p=2, k=48)
                for c in range(NCODE):
                    gc = gpool.tile([P, ROW], f16)
                    gi = nc.gpsimd.indirect_dma_start(
                        out=gc[:],
                        out_offset=None,
                        in_=tab_t[:],
                        in_offset=bass.IndirectOffsetOnAxis(
                            ap=idx[:, c : c + 1], axis=0
                        ),
                    )
                    if N_SWDGE_QUEUES > 1 and c % N_SWDGE_QUEUES:
                        gi.ins.queue = f"qPoolDynamic{c % N_SWDGE_QUEUES}"
                    gv = gc[:].rearrange("n (p k c) -> n p k c", p=2, k=48)
                    ch0, wdt, col0 = _code_map(c)
                    # wide codes ride ACT, except the last one: its consumer
                    # is on the group-exit critical path and DVE is ~2x faster
                    if 14 <= c < NCODE - 1:
                        nc.scalar.activation(
                            out=od4[:, :, :, col0 : col0 + wdt],
                            in_=gv[:, :, :, ch0 : ch0 + wdt],
                            func=mybir.ActivationFunctionType.Identity,
                            bias=tt[:, c : c + 1],
                            scale=sg[:, c : c + 1],
                        )
                    else:
                        nc.vector.tensor_scalar(
                            out=od4[:, :, :, col0 : col0 + wdt],
                            in0=gv[:, :, :, ch0 : ch0 + wdt],
                            scalar1=sg[:, c : c + 1],
                            scalar2=tt[:, c : c + 1],
                            op0=mybir.AluOpType.mult,
                            op1=mybir.AluOpType.add,
                        )
                # all stores on the (otherwise idle) sync engine so they never
                # queue behind the scalar engine's activation backlog
                nc.sync.dma_start(out=out_t[b0 : b0 + P, :], in_=od[:])
    nc.compile()
    return nc


def make_weights():
    w = np.tile((2.0 ** np.arange(NBITS)).astype(np.float32), NCODE)
    return np.broadcast_to(w, (P, NCODE * NBITS)).copy()


def make_in_maps(x, table):
    tab = np.ascontiguousarray(table.reshape(L, ROW))
    w = make_weights()
    return [
        {
            "x": np.ascontiguousarray(x[i * BC : (i + 1) * BC]),
            "table": tab,
            "w": w,
        }
        for i in range(NCORES)
    ]


def assemble_output(parts):
    out = np.full((BATCH, 2, 126, 128), 0.5, dtype=np.float32)
    for i, p in enumerate(parts):
        out[i * BC : (i + 1) * BC, :, 19:67, :] = p.reshape(BC, 2, 48, 128)
    return out


_NC_CACHE = None


def _get_module():
    global _NC_CACHE
    if _NC_CACHE is None:
        _NC_CACHE = build_module()
    return _NC_CACHE


def kernel(x: np.ndarray, table: np.ndarray) -> np.ndarray:
    from concourse.bass_utils import run_bass_kernel_spmd

    x = np.asarray(x)
    table = np.asarray(table)
    assert x.shape == (BATCH, XCOLS) and table.shape == (L, 2, 48, 8)
    nc = _get_module()
    res = run_bass_kernel_spmd(nc, make_in_maps(x, table), core_ids=list(range(NCORES)))
    return assemble_output([res.results[i]["out"] for i in range(NCORES)])


# revision 9
# speedup vs baseline: 1.0324x; 1.0182x over previous
"""v3 fallback: per-code indirect gathers (one offset per partition — the
lowering the HW ucode verifiably supports), fp16 data-only device output,
host-side 0.5-fill + fp32 upcast during unshard.
"""

import numpy as np

import concourse.bacc as bacc
import concourse.bass as bass
import concourse.mybir as mybir
import concourse.tile as tile

BATCH = 8192
XCOLS = 512
NCODE = 23
NBITS = 22
L = 131072
ROW = 2 * 48 * 8     # 768 fp16 = 1536B per table row
NCORES = 8
BC = BATCH // NCORES
P = 128
GROUPS = BC // P
DROW = 2 * 48 * 128  # 12288

f16 = mybir.dt.float16
f32 = mybir.dt.float32
i32 = mybir.dt.int32

N_SWDGE_QUEUES = 2


def _code_map(c):
    if c < 7:
        return 0, 4, c * 8
    if c < 14:
        return 4, 4, (c - 7) * 8 + 4
    return 0, 8, (c - 7) * 8


def build_module():
    nc = bacc.Bacc(
        "TRN2", target_bir_lowering=False, debug=False,
        num_swdge_queues=N_SWDGE_QUEUES,
    )
    x_t = nc.dram_tensor("x", [BC, XCOLS], i32, kind="ExternalInput")
    tab_t = nc.dram_tensor("table", [L, ROW], f16, kind="ExternalInput")
    w_t = nc.dram_tensor("w", [P, NCODE * NBITS], f32, kind="ExternalInput")
    out_t = nc.dram_tensor("out", [BC, DROW], f16, kind="ExternalOutput")

    with tile.TileContext(nc) as tc:
        with (
            tc.tile_pool(name="const", bufs=1) as cpool,
            tc.tile_pool(name="xp", bufs=2) as xpool,
            tc.tile_pool(name="sm", bufs=GROUPS) as spool,
            tc.tile_pool(name="gt", bufs=16) as gpool,
            tc.tile_pool(name="op", bufs=2) as opool,
        ):
            w_tile = cpool.tile([P, NCODE * NBITS], f32)
            nc.sync.dma_start(w_tile[:], w_t[:])

            idxs, tts, sgs = [], [], []
            for g in range(GROUPS):
                b0 = g * P
                x_tile = xpool.tile([P, XCOLS], i32)
                nc.sync.dma_start(x_tile[:], x_t[b0 : b0 + P, :])
                xf = xpool.tile([P, XCOLS], f32)
                nc.vector.tensor_copy(out=xf[:], in_=x_tile[:])
                prod = xpool.tile([P, NCODE * NBITS], f32)
                nc.vector.tensor_tensor(
                    out=prod[:], in0=xf[:, 6:], in1=w_tile[:],
                    op=mybir.AluOpType.mult,
                )
                codes = spool.tile([P, NCODE], f32, tag="codes")
                nc.vector.tensor_reduce(
                    out=codes[:],
                    in_=prod[:].rearrange("n (c a) -> n c a", a=NBITS),
                    axis=mybir.AxisListType.X,
                    op=mybir.AluOpType.add,
                )
                codesi = spool.tile([P, NCODE], i32, tag="codesi")
                nc.vector.tensor_copy(out=codesi[:], in_=codes[:])
                idx = spool.tile([P, NCODE], i32, tag="idx")
                nc.vector.tensor_scalar(
                    out=idx[:], in0=codesi[:],
                    scalar1=L - 1, scalar2=None,
                    op0=mybir.AluOpType.bitwise_and,
                )
                tt = spool.tile([P, NCODE], f32, tag="tt")
                nc.vector.tensor_scalar(
                    out=tt[:], in0=codes[:],
                    scalar1=float(L), scalar2=None,
                    op0=mybir.AluOpType.is_gt,
                )
                sg = spool.tile([P, NCODE], f32, tag="sg")
                nc.vector.tensor_scalar(
                    out=sg[:], in0=tt[:],
                    scalar1=-2.0, scalar2=1.0,
                    op0=mybir.AluOpType.mult, op1=mybir.AluOpType.add,
                )
                idxs.append(idx); tts.append(tt); sgs.append(sg)

            for g in range(GROUPS):
                b0 = g * P
                idx, tt, sg = idxs[g], tts[g], sgs[g]
                od = opool.tile([P, DROW], f16)
                od4 = od[:].rearrange("n (p k c) -> n p k c", p=2, k=48)
                for c in range(NCODE):
                    gc = gpool.tile([P, ROW], f16)
                    gi = nc.gpsimd.indirect_dma_start(
                        out=gc[:],
                        out_offset=None,
                        in_=tab_t[:],
                        in_offset=bass.IndirectOffsetOnAxis(
                            ap=idx[:, c : c + 1], axis=0
                        ),
                    )
                    if N_SWDGE_QUEUES > 1 and c % N_SWDGE_QUEUES:
                        gi.ins.queue = f"qPoolDynamic{c % N_SWDGE_QUEUES}"
                    gv = gc[:].rearrange("n (p k c) -> n p k c", p=2, k=48)
                    ch0, wdt, col0 = _code_map(c)
                    if c >= 14:
                        nc.scalar.activation(
                            out=od4[:, :, :, col0 : col0 + wdt],
                            in_=gv[:, :, :, ch0 : ch0 + wdt],
                            func=mybir.ActivationFunctionType.Identity,
                            bias=tt[:, c : c + 1],
                            scale=sg[:, c : c + 1],
                        )
                    else:
                        nc.vector.tensor_scalar(
                            out=od4[:, :, :, col0 : col0 + wdt],
                            in0=gv[:, :, :, ch0 : ch0 + wdt],
                            scalar1=sg[:, c : c + 1],
                            scalar2=tt[:, c : c + 1],
                            op0=mybir.AluOpType.mult,
                            op1=mybir.AluOpType.add,
                        )
                eng = nc.sync if g % 2 == 0 else nc.scalar
                eng.dma_start(out=out_t[b0 : b0 + P, :], in_=od[:])
    nc.compile()
    return nc


def make_weights():
    w = np.tile((2.0 ** np.arange(NBITS)).astype(np.float32), NCODE)
    return np.broadcast_to(w, (P, NCODE * NBITS)).copy()


def make_in_maps(x, table):
    tab = np.ascontiguousarray(table.reshape(L, ROW))
    w = make_weights()
    return [
        {
            "x": np.ascontiguousarray(x[i * BC : (i + 1) * BC]),
            "table": tab,
            "w": w,
        }
        for i in range(NCORES)
    ]


def assemble_output(parts):
    out = np.full((BATCH, 2, 126, 128), 0.5, dtype=np.float32)
    for i, p in enumerate(parts):
        out[i * BC : (i + 1) * BC, :, 19:67, :] = p.reshape(BC, 2, 48, 128)
    return out


_NC_CACHE = None


def _get_module():
    global _NC_CACHE
    if _NC_CACHE is None:
        _NC_CACHE = build_module()
    return _NC_CACHE


def kernel(x: np.ndarray, table: np.ndarray) -> np.ndarray:
    from concourse.bass_utils import run_bass_kernel_spmd

    x = np.asarray(x)
    table = np.asarray(table)
    assert x.shape == (BATCH, XCOLS) and table.shape == (L, 2, 48, 8)
    nc = _get_module()
    res = run_bass_kernel_spmd(nc, make_in_maps(x, table), core_ids=list(range(NCORES)))
    return assemble_output([res.results[i]["out"] for i in range(NCORES)])
